# revision 12
# baseline (speedup 1.0000x reference)
"""Distributed Bass kernel: fused multi-head attention block on 8 TRN2 NeuronCores.

Problem: x[2,2048,1024] -> QKV proj -> RoPE(q,k) -> softmax(q k^T/8) v -> out proj.

Sharding: tensor-parallel over heads. 16 heads / 8 cores = 2 heads per core.
Each core computes QKV for its 2 heads (full sequence), RoPE, attention, then
an AllToAll converts head-sharding to token-sharding so the output projection
runs against the FULL Wout with no AllReduce. Host reassembles the token
slices (interleaved mapping, see below).

Pipeline design (v3):
 - Scores matmuls are K=64 row-tiled: head A contracts on array rows 0-63,
   head B on rows 64-127, running CONCURRENTLY on the PE (tile_position auto-
   derived from base partitions). No zero-padded q copies needed.
 - One ACTIVATE(exp) per key-tile covers both heads ([128,1024] PSUM ->
   [A|B] bf16 SBUF). ScalarE's ~145us of exp is the phase-2 budget;
   everything else is structured to hide under it.
 - QKV chunks are emitted interleaved with attention groups so the PE/DMA
   work of batch 1's projection hides under batch 0's exp shadow. DMA issue
   is one instruction per chunk (3D access patterns) since each dma_start
   costs its issuing engine ~600ns serially.
 - Bulk loads issue from the sync engine's HW DGE queues; collective staging
   and gathers issue from GpSimd so the two queues run in parallel.
 - The AllToAll is split into 8 per-chunk pieces (tokens interleaved across
   cores). Output projection m-tiles are emitted TWO chunks after their
   gather was kicked off so the in-order Tensor queue never waits on
   collective latency; the last chunk zippers its own PV into its score
   groups to shorten the serial tail.

Token mapping: attention chunk j (512 tokens, j = b*4+qc) sends its 64-token
group g to core g. Core c's output rows tau = j*64 + i correspond to global
token j*512 + c*64 + i. The host unshard reorders accordingly.

Compute dtype bf16 (PE 1 cycle/row), f32 PSUM accumulation. Softmax skips the
max-subtraction (scores ~N(0,2), |s|<~12, exp safe in f32) and folds the
denominator into the PV matmul via a ones-column appended to v.
"""

import sys

for _p in ("/opt/trn_rl_repo", "/root/.axon_site/_ro/trn_rl_repo"):
    if _p not in sys.path:
        sys.path.append(_p)

import numpy as np
import ml_dtypes

B, N, HID = 2, 2048, 1024
H, DH = 16, 64
NCORES = 8
HPC = H // NCORES          # heads per core = 2
T = B * N                  # 4096 flattened tokens
TS = T // NCORES           # 512 tokens per core after AllToAll
EPC = HPC * DH             # 128 features per core
CH = 512                   # token chunk for QKV phase
NCH = T // CH              # 8 chunks
KT = 128                   # key tile
QC = 512                   # query chunk in attention
NKT = N // KT              # 16 key tiles per batch
NJ = NCH                   # 8 attention chunks (b,qc)
GRP = 64                   # tokens per destination core per chunk

_bf16 = ml_dtypes.bfloat16


def _build_graph():
    import concourse.bass as bass
    import concourse.mybir as mybir
    import concourse.tile as tile
    from concourse import bacc

    f32 = mybir.dt.float32
    bf16 = mybir.dt.bfloat16

    nc = bacc.Bacc("TRN2", target_bir_lowering=False, debug=False, num_devices=NCORES)

    xT_e = nc.declare_dram_parameter("xT", [HID, T], bf16, isOutput=False)
    wqkvT_e = nc.declare_dram_parameter("wqkvT", [HID, 3 * EPC], bf16, isOutput=False)
    woutT_e = nc.declare_dram_parameter("woutT", [HID, HID], bf16, isOutput=False)
    cos2_e = nc.declare_dram_parameter("cos2", [2 * DH, T], bf16, isOutput=False)
    sin2_e = nc.declare_dram_parameter("sin2", [2 * DH, T], bf16, isOutput=False)
    perm_e = nc.declare_dram_parameter("perm", [128, 128], bf16, isOutput=False)
    ident_e = nc.declare_dram_parameter("ident", [128, 128], bf16, isOutput=False)
    out_e = nc.declare_dram_parameter("out", [TS, HID], f32, isOutput=True)

    with tile.TileContext(nc) as tc:
        with (
            tc.tile_pool(name="const", bufs=1) as cpool,
            tc.tile_pool(name="work", bufs=1) as wpool,
            tc.tile_pool(name="stream", bufs=4) as spool,
            tc.tile_pool(name="psum", bufs=2, space="PSUM") as pspool,
            tc.tile_pool(name="dram", bufs=1, space="DRAM") as dpool,
        ):
            # ---- constants / weights, split across issuing engines ----
            wqkvT = cpool.tile([128, 8 * 3 * EPC], bf16)       # 8 k-tiles side by side
            wq3o = wqkvT.rearrange("p (kt c) -> p kt c", kt=8)
            wq3i = wqkvT_e[:, :].rearrange("(kt p) c -> p kt c", p=128)
            nc.sync.dma_start(wq3o[:, 0:4, :], wq3i[:, 0:4, :])
            nc.scalar.dma_start(wq3o[:, 4:8, :], wq3i[:, 4:8, :])
            perm = cpool.tile([128, 128], bf16)
            ident = cpool.tile([128, 128], bf16)
            woutT = cpool.tile([128, 8 * HID], bf16)
            cos2 = cpool.tile([128, T], bf16)
            sin2 = cpool.tile([128, T], bf16)
            nc.gpsimd.dma_start(perm[:, :], perm_e[:, :])
            nc.gpsimd.dma_start(ident[:, :], ident_e[:, :])

            # ---- persistent working tensors ----
            q_sb = wpool.tile([128, T], bf16)      # roped q (A rows 0-63, B 64-127)
            k_sb = wpool.tile([128, T], bf16)      # roped k
            vT_sb = wpool.tile([128, T], bf16)     # v transposed [e, t]
            # per-key-tile v tables: [vA(64) | onesA | vB(64) | onesB] = 130 cols
            vex = wpool.tile([128, 32 * 130], bf16)
            vex3 = vex.rearrange("p (s c) -> p s c", c=130)
            nc.vector.memset(vex3[:, :, DH:DH + 1], 1.0)
            nc.vector.memset(vex3[:, :, 129:130], 1.0)

            # ================= emission helpers =================
            xs_tiles = {}

            def qkv_dma(c, eng=None):
                """One DMA bringing the whole x chunk c (8 k-tiles, 1 MB)."""
                xt = spool.tile([128, 8 * CH], bf16, name="xt", tag="xs", bufs=3)
                (eng or nc.sync).dma_start(
                    xt.rearrange("p (kt t) -> p kt t", kt=8)[:, :, :],
                    xT_e[:, c * CH:(c + 1) * CH].rearrange(
                        "(kt p) t -> p kt t", p=128),
                )
                xs_tiles[c] = xt

            def qkv_group(c, which, dest):
                xt = xs_tiles[c]
                sl = slice(c * CH, (c + 1) * CH)
                ps = pspool.tile([128, 2 * QC], f32, tag="sc", bufs=2)
                for kt in range(8):
                    nc.tensor.matmul(
                        ps[:, 0:CH],
                        wqkvT[:, kt * 3 * EPC + which * EPC:
                              kt * 3 * EPC + (which + 1) * EPC],
                        xt[:, kt * CH:(kt + 1) * CH],
                        start=(kt == 0),
                        stop=(kt == 7),
                    )
                nc.vector.tensor_copy(dest[:, sl], ps[:, 0:CH])
                if which == 2:
                    xs_tiles.pop(c)

            def rope(c, srd):
                sl = slice(c * CH, (c + 1) * CH)
                pps = pspool.tile([128, 2 * QC], f32, tag="sc", bufs=2)
                nc.tensor.matmul(
                    pps[:, 0:CH], perm[:, :], srd[:, sl],
                    start=True, stop=True,
                )
                tmp = spool.tile([128, CH], bf16, tag="ropetmp", bufs=2)
                nc.vector.tensor_mul(tmp[:, :], pps[:, 0:CH], sin2[:, sl])
                nc.vector.tensor_mul(srd[:, sl], srd[:, sl], cos2[:, sl])
                nc.vector.tensor_add(srd[:, sl], srd[:, sl], tmp[:, :])

            def vtrans(c):
                for tt in range(CH // 128):
                    slot = c * (CH // 128) + tt
                    tp = pspool.tile([128, 2 * QC], bf16, tag="sc", bufs=2)
                    nc.tensor.transpose(
                        tp[:, 0:128],
                        vT_sb[:, c * CH + tt * 128:c * CH + (tt + 1) * 128],
                        ident[:, :],
                    )
                    nc.vector.tensor_copy(vex3[:, slot, 0:DH], tp[:, 0:DH])
                    nc.vector.tensor_copy(vex3[:, slot, DH + 1:2 * DH + 1],
                                          tp[:, DH:2 * DH])

            def qkv_piece(c, p):
                """Piece A: q proj + rope-q; B: k proj + rope-k; C: v + transpose."""
                if p == 0:
                    qkv_group(c, 0, q_sb)
                    rope(c, q_sb)
                elif p == 1:
                    qkv_group(c, 1, k_sb)
                    rope(c, k_sb)
                else:
                    qkv_group(c, 2, vT_sb)
                    vtrans(c)

            def qkv_compute(c):
                for p in range(3):
                    qkv_piece(c, p)

            def score_group(j, kt, expT):
                """Scores + exp for attention chunk j, key tile kt (both heads)."""
                b, qc = j // 4, j % 4
                q0 = b * N + qc * QC
                k0 = b * N + kt * KT
                sps = pspool.tile([128, 2 * QC], f32, tag="sc", bufs=2)
                nc.tensor.matmul(
                    sps[:, 0:QC],
                    k_sb[0:DH, k0:k0 + KT],
                    q_sb[0:DH, q0:q0 + QC],
                    start=True, stop=True,
                )
                nc.tensor.matmul(
                    sps[:, QC:2 * QC],
                    k_sb[DH:128, k0:k0 + KT],
                    q_sb[DH:128, q0:q0 + QC],
                    start=True, stop=True,
                )
                nc.scalar.activation(
                    expT[:, kt * 2 * QC:(kt + 1) * 2 * QC],
                    sps[:, :],
                    mybir.ActivationFunctionType.Exp,
                    scale=DH ** -0.5,
                )

            def pv_pair(st, pair):
                """PV matmuls for key tiles 2*pair, 2*pair+1 of a chunk."""
                (j, opsAB, expT) = st
                b = j // 4
                for kt in (2 * pair, 2 * pair + 1):
                    slot = b * (N // 128) + kt
                    nc.tensor.matmul(
                        opsAB[0][0:DH + 1, :],
                        vex3[:, slot, 0:DH + 1],
                        expT[:, kt * 2 * QC:kt * 2 * QC + QC],
                        start=(kt == 0),
                        stop=(kt == NKT - 1),
                    )
                    nc.tensor.matmul(
                        opsAB[1][0:DH + 1, :],
                        vex3[:, slot, DH + 1:2 * (DH + 1)],
                        expT[:, kt * 2 * QC + QC:(kt + 1) * 2 * QC],
                        start=(kt == 0),
                        stop=(kt == NKT - 1),
                    )

            ovs = {}

            def normalize(st):
                """Divide PV by the folded denominator row; write ovT chunk."""
                (j, opsAB, expT) = st
                ovT = spool.tile([128, QC], bf16, tag="ov", bufs=3)
                for h in range(HPC):
                    ops = opsAB[h]
                    den = spool.tile([1, QC], f32, tag="den", bufs=2)
                    nc.vector.tensor_copy(den[0:1, :], ops[DH:DH + 1, :])
                    rec = spool.tile([1, QC], f32, tag="rec", bufs=2)
                    nc.vector.reciprocal_approx_fast(rec[0:1, :], den[0:1, :])
                    bcs = spool.tile([64, QC], f32, tag="bcs", bufs=2)
                    nc.gpsimd.partition_broadcast(bcs[:, :], rec[0:1, :])
                    nc.vector.tensor_mul(
                        ovT[h * DH:(h + 1) * DH, :], ops[0:DH, :], bcs[:, :]
                    )
                ovs[j] = ovT

            # per-chunk collective staging
            a2a_ins = [dpool.tile([NCORES * 128, GRP], bf16,
                                  name=f"a2ai{j}", tag=f"a2ai{j}")
                       for j in range(NJ)]
            a2a_outs = [dpool.tile([NCORES * 128, GRP], bf16,
                                   name=f"a2ao{j}", tag=f"a2ao{j}")
                        for j in range(NJ)]
            # gathered tiles: gTm[m] holds all 1024 features for output m-tile m
            gTm = [wpool.tile([128, NCORES * 128], bf16, name=f"gt{m}",
                              tag=f"gt{m}")
                   for m in range(4)]

            def a2a_chunk(j):
                """Ship ovT chunk j through the per-chunk AllToAll and gather."""
                ain3 = a2a_ins[j].rearrange("(g p) t -> p g t", p=128)
                ov3 = ovs[j].rearrange("p (g t) -> p g t", g=NCORES)
                nc.gpsimd.dma_start(ain3[:, :, :], ov3[:, :, :])
                nc.gpsimd.collective_compute(
                    "AllToAll",
                    mybir.AluOpType.bypass,
                    ins=[a2a_ins[j].opt()],
                    outs=[a2a_outs[j].opt()],
                    replica_groups=[list(range(NCORES))],
                )
                aout3 = a2a_outs[j].rearrange("(e p) t -> p e t", p=128)
                m, half = j // 2, j % 2
                g3 = gTm[m].rearrange("p (e t) -> p e t", t=128)
                nc.gpsimd.dma_start(g3[:, :, half * GRP:(half + 1) * GRP],
                                    aout3[:, :, :])

            def outproj(m, rows=(0, 128)):
                """Output projection for token rows [rows) of m-tile m."""
                g3 = gTm[m]
                r0, r1 = rows
                for nn in range(HID // 512):
                    odps = pspool.tile([128, 2 * QC], f32, tag="sc", bufs=2)
                    for et in range(8):
                        nc.tensor.matmul(
                            odps[r0:r1, 0:512],
                            g3[:, et * 128 + r0:et * 128 + r1],
                            woutT[:, et * HID + nn * 512:et * HID + (nn + 1) * 512],
                            start=(et == 0),
                            stop=(et == 7),
                        )
                    osb = spool.tile([128, 512], f32, tag="osb", bufs=2)
                    nc.vector.tensor_copy(osb[r0:r1, :], odps[r0:r1, 0:512])
                    nc.sync.dma_start(
                        out_e[m * 128 + r0:m * 128 + r1,
                              nn * 512:(nn + 1) * 512],
                        osb[r0:r1, :],
                    )

            # ================= emission schedule =================
            # Attention chunk j's key tile kt needs QKV chunk (j//4)*4 + kt//4.
            nc.gpsimd.dma_start(cos2[:, 0:CH], cos2_e[:, 0:CH])
            nc.gpsimd.dma_start(sin2[:, 0:CH], sin2_e[:, 0:CH])
            xt0 = spool.tile([128, 8 * CH], bf16, name="xt0", tag="xs", bufs=3)
            xt03 = xt0.rearrange("p (kt t) -> p kt t", kt=8)
            xi03 = xT_e[:, 0:CH].rearrange("(kt p) t -> p kt t", p=128)
            nc.sync.dma_start(xt03[:, 0:4, :], xi03[:, 0:4, :])
            nc.scalar.dma_start(xt03[:, 4:8, :], xi03[:, 4:8, :])
            xs_tiles[0] = xt0
            qkv_dma(1, nc.scalar)
            # rest of the rope factors + Wout, off the critical path
            nc.gpsimd.dma_start(cos2[:, CH:T], cos2_e[:, CH:T])
            nc.gpsimd.dma_start(sin2[:, CH:T], sin2_e[:, CH:T])
            nc.gpsimd.dma_start(
                woutT.rearrange("p (kt c) -> p kt c", kt=8)[:, :, :],
                woutT_e[:, :].rearrange("(kt p) c -> p kt c", p=128),
            )
            qkv_compute(0)

            # emission hooks: before group `pair` of attn chunk j
            dma_at = {
                0: {0: 2, 2: 3, 8: 4},
                1: {0: 5},
                2: {0: 6, 3: 7},
            }
            cmp_at = {
                0: {1: 1, 3: 2, 5: 3},
                1: {2: 4},
                2: {2: 5, 5: 6},
                3: {2: 7},
            }
            # outproj(m) needs gathers of chunks 2m, 2m+1; emit two chunks
            # later so Tensor never waits on the collective in-order
            oproj_at = {3: 0, 5: 1, 7: 2}

            pending = None
            for j in range(NJ):
                expT = spool.tile([128, NKT * 2 * QC], bf16, name="expT",
                                  tag="expT", bufs=2)
                # chunks 6 and 7 zipper their own PV so their collectives can
                # fire early and the final outproj splits off the tail
                selfzip = (j >= NJ - 2)
                if selfzip:
                    cur = (j,
                           (pspool.tile([128, QC], f32, name="opsA",
                                        tag="pv", bufs=4),
                            pspool.tile([128, QC], f32, name="opsB",
                                        tag="pv", bufs=4)),
                           expT)
                for pair in range(NKT // 2):
                    c = dma_at.get(j, {}).get(pair)
                    if c is not None:
                        qkv_dma(c)
                    for (pc, pp) in piece_at.get(j, {}).get(pair, ()):
                        qkv_piece(pc, pp)
                    if pair == 6 and j in oproj_at:
                        outproj(oproj_at[j])
                    if j == NJ - 1 and pair == 5:
                        outproj(3, rows=(0, 64))
                    score_group(j, 2 * pair, expT)
                    score_group(j, 2 * pair + 1, expT)
                    if pending is not None:
                        pv_pair(pending, pair)
                        if pair == NKT // 2 - 1:
                            normalize(pending)
                            a2a_chunk(pending[0])
                    if selfzip and pair >= 1:
                        pv_pair(cur, pair - 1)
                if selfzip:
                    pv_pair(cur, NKT // 2 - 1)
                    normalize(cur)
                    a2a_chunk(j)
                    pending = None
                else:
                    opsAB = (pspool.tile([128, QC], f32, name="opsA",
                                         tag="pv", bufs=4),
                             pspool.tile([128, QC], f32, name="opsB",
                                         tag="pv", bufs=4))
                    pending = (j, opsAB, expT)
            outproj(3, rows=(64, 128))

    nc.finalize()
    return nc


def _host_inputs(x, rope, Wqkv, Wout):
    """Build the 8 per-core input maps with host-side layout prep."""
    xf = np.ascontiguousarray(x.reshape(T, HID).T).astype(_bf16)        # [1024, 4096]
    woutT = np.ascontiguousarray(Wout.T).astype(_bf16)                  # [1024, 1024]

    rf = rope.reshape(T, DH)                                            # [4096, 64]
    cosE = np.repeat(rf[:, 0::2], 2, axis=1).T                          # [64, 4096]
    sinE = np.repeat(rf[:, 1::2], 2, axis=1).T
    sgn = np.where(np.arange(DH) % 2 == 0, -1.0, 1.0)[:, None]
    sinS = (sinE * sgn)
    cos2 = np.ascontiguousarray(np.concatenate([cosE, cosE], 0)).astype(_bf16)
    sin2 = np.ascontiguousarray(np.concatenate([sinS, sinS], 0)).astype(_bf16)

    pm = np.zeros((128, 128), np.float32)
    for d in range(128):
        pm[d ^ 1, d] = 1.0       # partner[d] = q[d^1]; lhsT = S (symmetric)
    perm = pm.astype(_bf16)
    ident = np.eye(128, dtype=np.float32).astype(_bf16)

    w3 = Wqkv.reshape(3, H, DH, HID)
    in_maps = []
    for c in range(NCORES):
        blocks = []
        for which in range(3):
            for hl in range(HPC):
                blocks.append(w3[which, 2 * c + hl])                    # [64, 1024]
        wq = np.concatenate(blocks, 0)                                  # [384, 1024]
        wqkvT = np.ascontiguousarray(wq.T).astype(_bf16)                # [1024, 384]
        in_maps.append({
            "xT": xf, "wqkvT": wqkvT, "woutT": woutT,
            "cos2": cos2, "sin2": sin2, "perm": perm, "ident": ident,
        })
    return in_maps


_CACHE = {}


def kernel(x, rope, Wqkv, Wout):
    from concourse.bass_utils import run_bass_kernel_spmd

    if "nc" not in _CACHE:
        _CACHE["nc"] = _build_graph()
    nc = _CACHE["nc"]
    in_maps = _host_inputs(np.asarray(x, np.float32), np.asarray(rope, np.float32),
                           np.asarray(Wqkv, np.float32), np.asarray(Wout, np.float32))
    res = run_bass_kernel_spmd(nc, in_maps, core_ids=list(range(NCORES)))
    # core c row tau = j*64 + i  ->  global token j*512 + c*64 + i
    full = np.empty((T, HID), np.float32)
    for c in range(NCORES):
        part = np.asarray(res.results[c]["out"], np.float32)            # [512, 1024]
        p3 = part.reshape(NJ, GRP, HID)
        for j in range(NJ):
            full[j * CH + c * GRP:(j * CH) + (c + 1) * GRP] = p3[j]
    return full.reshape(B, N, HID)
